# revision 12
# baseline (speedup 1.0000x reference)
"""Distributed Trainium2 Bass kernel for nn_Attention (dense transformer block).

Reference computation (full shapes):
    x: [2, 2048, 1024]
    xn = LayerNorm(x, gamma, beta)
    q = xn @ w_q ; k, v = split(xn @ w_kv)   (16 heads, head dim 64)
    attn = softmax(q k^T / 8) v  over seq 2048
    out = attn_out @ w_out + b_out           -> [2, 2048, 1024]

Sharding over 8 NeuronCores (head tensor-parallel, 2 heads/core; rows of the
flattened [4096, 1024] activations sharded 512/core for LayerNorm/out-proj).
All 8 cores run one identical SPMD graph.

v3 design (bf16 numerics; fp8 tested and rejected — each fp8 matmul operand
costs 2.7-4% output rel err vs the 2e-2 gate):
  * Dummy AllGather at t=0 absorbs the NRT first-collective barrier +
    CC-trigger wake latency concurrent with the LayerNorm front-end.
  * x DMAs issued first on the scalar queue; params on gpsimd behind the
    dummy doorbell; weight loads deferred past the real AllGather doorbell.
  * LayerNorm -> PE transpose -> gamma/beta -> xn^T bf16, bounced to DRAM
    per ko-chunk so the AllGather doorbell fires ~17us in.
  * Attention: flash over 16 j-tiles per (batch, 512-i-chunk); both heads'
    score matmuls overlap in the PE array (disjoint row groups); exp on ACT
    straight from PSUM; attn@v with a 64-wide ones block giving denominator
    replicas; softmax normalize via reciprocal_approx_fast (inputs must be
    partition-base-0: custom DVE ops corrupt on shifted bases).
  * QKV for later row-blocks woven into attention emission as fine-grained
    tasks; PSUM drains on DVE (not ACT) to keep the exp stream dense.
  * attn-out columns bounced to DRAM eagerly per unit; AllToAll fires
    immediately after the last unit; out-proj + bias add + store chunked.
"""

import numpy as np

import concourse.bass as bass
import concourse.mybir as mybir
import concourse.tile as tile
from concourse import bacc
from concourse.bass_utils import run_bass_kernel_spmd

F32 = mybir.dt.float32
BF16 = mybir.dt.bfloat16
FP8 = mybir.dt.float8e4
AF = mybir.ActivationFunctionType
ALU = mybir.AluOpType

N_CORES = 8
DIM = 1024
N = 2048  # sequence length
R = 4096  # total rows (2 batches x 2048)
RL = 512  # rows per core
H_LOC = 2  # heads per core
DH = 64
CH = H_LOC * DH  # 128 channels per core
SCALE = DH**-0.5
KO = DIM // 128  # 8 contraction chunks
GROUPS = [list(range(N_CORES))]


def build_nc():
    nc = bacc.Bacc("TRN2", target_bir_lowering=False, debug=False, num_devices=N_CORES)

    x_ext = nc.declare_dram_parameter("x", [RL, DIM], F32, isOutput=False)
    wq_ext = nc.declare_dram_parameter("wq", [DIM, CH], F32, isOutput=False)
    wk_ext = nc.declare_dram_parameter("wk", [DIM, CH], F32, isOutput=False)
    wv_ext = nc.declare_dram_parameter("wv", [DIM, CH], F32, isOutput=False)
    wo_ext = nc.declare_dram_parameter("wo", [DIM, DIM], F32, isOutput=False)
    gamma_ext = nc.declare_dram_parameter("gamma", [DIM], F32, isOutput=False)
    beta_ext = nc.declare_dram_parameter("beta", [DIM], F32, isOutput=False)
    bias_ext = nc.declare_dram_parameter("bias", [DIM], F32, isOutput=False)
    out_ext = nc.declare_dram_parameter("out", [RL, DIM], F32, isOutput=True)

    # DRAM bounce buffers for collectives
    dummy_in = nc.dram_tensor("dummy_in", [64], FP8)
    dummy_out = nc.dram_tensor("dummy_out", [N_CORES * 64], FP8, addr_space="Shared")
    xnT_bounce = nc.dram_tensor("xnT_bounce", [DIM, RL], BF16)
    xnT_gath = nc.dram_tensor(
        "xnT_gath", [N_CORES * DIM, RL], BF16, addr_space="Shared"
    )
    ao_bounce = nc.dram_tensor("ao_bounce", [N_CORES, CH, RL], BF16)
    ao_recv = nc.dram_tensor("ao_recv", [N_CORES, CH, RL], BF16)

    with tile.TileContext(nc) as tc:
        with (
            tc.tile_pool(name="singles", bufs=1) as singles,
            tc.tile_pool(name="temps", bufs=3) as temps,
            tc.tile_pool(name="small", bufs=4) as small,
            tc.tile_pool(name="etile", bufs=3) as epool,
            tc.tile_pool(name="xnrp", bufs=8) as xnrp,
            tc.tile_pool(name="psum", bufs=2, space="PSUM") as psum,
        ):
            # ---- dummy collective first: pays the NRT collective-init
            # barrier + CC wake latency while the LN front-end runs ----
            nc.gpsimd.collective_compute(
                "AllGather",
                ALU.bypass,
                ins=[dummy_in[:]],
                outs=[dummy_out[:]],
                replica_groups=GROUPS,
            )

            # ---- x DMAs first on scalar: LayerNorm input is the critical path
            x_tiles = []
            for t in range(4):
                x_t = temps.tile([128, DIM], F32, tag="x", bufs=4)
                nc.scalar.dma_start(out=x_t[:], in_=x_ext[t * 128 : (t + 1) * 128, :])
                x_tiles.append(x_t)

            # constants / params on gpsimd (behind the dummy doorbell)
            import ml_dtypes

            ident_const = nc.inline_tensor(
                np.eye(128, dtype=ml_dtypes.bfloat16), name="ident_const"
            )
            ident = singles.tile([128, 128], BF16, tag="ident")
            nc.gpsimd.dma_start(out=ident[:], in_=ident_const.ap())
            gamma_sb = singles.tile([128, KO], F32, tag="gamma")
            nc.gpsimd.dma_start(
                out=gamma_sb[:], in_=gamma_ext.ap().rearrange("(ko p) -> p ko", p=128)
            )
            beta_sb = singles.tile([128, KO], F32, tag="beta")
            nc.gpsimd.dma_start(
                out=beta_sb[:], in_=beta_ext.ap().rearrange("(ko p) -> p ko", p=128)
            )
            eps_sb = singles.tile([128, 1], F32, tag="eps")
            nc.vector.memset(eps_sb[:], 1e-5)

            # ---- Phase 1: LayerNorm on own 512 rows -> xc bf16 ----
            xc = []
            for t in range(4):
                x_t = x_tiles[t]
                st6 = small.tile([128, 2, 6], F32, tag="st6")
                nc.vector.bn_stats(out=st6[:, 0, :], in_=x_t[:, 0:512])
                nc.vector.bn_stats(out=st6[:, 1, :], in_=x_t[:, 512:1024])
                mv = small.tile([128, 2], F32, tag="mv")
                nc.vector.bn_aggr(out=mv[:], in_=st6[:])
                sd = small.tile([128, 1], F32, tag="sd")
                nc.scalar.activation(
                    out=sd[:], in_=mv[:, 1:2], func=AF.Sqrt, bias=eps_sb[:], scale=1.0
                )
                istd = small.tile([128, 1], F32, tag="istd")
                nc.vector.reciprocal_approx_fast(out=istd[:], in_=sd[:])
                xc_t = singles.tile([128, DIM], BF16, tag=f"xc{t}", name=f"xc{t}")
                for hh in range(2):
                    nc.vector.tensor_scalar(
                        xc_t[:, hh * 512 : (hh + 1) * 512],
                        x_t[:, hh * 512 : (hh + 1) * 512],
                        mv[:, 0:1],
                        istd[:],
                        ALU.subtract,
                        ALU.mult,
                    )
                xc.append(xc_t)

            # ---- transpose + gamma/beta -> xn^T bf16; bounce per ko chunk ----
            xnT_sb = singles.tile([128, KO, RL], BF16, tag="xnT")
            for ko in range(KO):
                ptr = psum.tile([128, 4, 128], BF16, tag="mm")
                for t in range(4):
                    nc.tensor.transpose(
                        ptr[:, t, :], xc[t][:, ko * 128 : (ko + 1) * 128], ident[:]
                    )
                nc.vector.tensor_scalar(
                    xnT_sb[:, ko, :],
                    ptr.rearrange("p a b -> p (a b)"),
                    gamma_sb[:, ko : ko + 1],
                    beta_sb[:, ko : ko + 1],
                    ALU.mult,
                    ALU.add,
                )
                nc.scalar.dma_start(
                    out=xnT_bounce[ko * 128 : (ko + 1) * 128, :], in_=xnT_sb[:, ko, :]
                )

            # ---- Phase 2: AllGather xn^T (all 8 cores) ----
            nc.gpsimd.collective_compute(
                "AllGather",
                ALU.bypass,
                ins=[xnT_bounce[:]],
                outs=[xnT_gath[:]],
                replica_groups=GROUPS,
            )

            # ---- weight loads + bf16 casts during the AllGather window ----
            def load_weight_bf16(ext, cols, tag):
                wf = singles.tile([128, KO, cols], F32, tag="wf", name="wf")
                nc.scalar.dma_start(
                    out=wf[:], in_=ext.ap().rearrange("(ko p) m -> p ko m", p=128)
                )
                wb = singles.tile([128, KO, cols], BF16, tag=tag, name=tag)
                nc.scalar.activation(
                    out=wb.rearrange("p a b -> p (a b)"),
                    in_=wf.rearrange("p a b -> p (a b)"),
                    func=AF.Copy,
                )
                return wb

            wq_b = load_weight_bf16(wq_ext, CH, "wq")
            wk_b = load_weight_bf16(wk_ext, CH, "wk")
            wv_b = load_weight_bf16(wv_ext, CH, "wv")

            # preload the exp ACT table while the scalar engine is idle
            dume = small.tile([128, 1], F32, tag="dume")
            nc.scalar.activation(out=dume[:], in_=eps_sb[:], func=AF.Exp, scale=1.0)

            # w_out: gpsimd stages f32, vector casts to bf16
            wo_b = singles.tile([128, KO, DIM], BF16, tag="wo", name="wo")
            for ko in range(KO):
                wof = temps.tile([128, DIM], F32, tag="x", bufs=4, name="wstage")
                nc.gpsimd.dma_start(
                    out=wof[:], in_=wo_ext[ko * 128 : (ko + 1) * 128, :]
                )
                nc.vector.tensor_copy(out=wo_b[:, ko, :], in_=wof[:])

            # bias broadcast [128,DIM] (partition-stride-0 read from DRAM)
            bias_bc = singles.tile([128, DIM], F32, tag="bias_bc")
            nc.gpsimd.dma_start(
                out=bias_bc[:],
                in_=bass.AP(tensor=bias_ext, offset=0, ap=[[0, 128], [1, DIM]]),
            )

            # v3d: stationary operand for attn@v. [128p, 32 rowtiles, 2h, 128]:
            # d 0:64 = v rows, d 64:128 = ones (64 denominator replicas).
            v3d = singles.tile([128, 32, H_LOC, 2 * DH], BF16, tag="v3d")
            nc.vector.memset(v3d[:, :, :, DH : 2 * DH], 1.0)

            # keep PE clocked across the AllGather window (results unused)
            for _w in range(40):
                pmw = psum.tile([128, 512], F32, tag="mm", name="pm_warm_ag")
                nc.tensor.matmul(
                    pmw[:],
                    lhsT=xnT_sb[:, 0, 0:128],
                    rhs=xnT_sb[:, 1, :],
                    start=True,
                    stop=True,
                )

            # ---- Phase 3: QKV (own 2 heads) as weavable tasks ----
            qT = singles.tile([128, R], BF16, tag="qT")
            kT = singles.tile([128, R], BF16, tag="kT")
            xnrs = {}

            def t_xnr(r):
                def f():
                    xnr = xnrp.tile([128, KO, RL], BF16, tag="xnr", name="xnr")
                    nc.gpsimd.dma_start(
                        out=xnr[:],
                        in_=xnT_gath[r * DIM : (r + 1) * DIM, :].rearrange(
                            "(ko p) lr -> p ko lr", p=128
                        ),
                    )
                    xnrs[r] = xnr

                return f

            def t_kq(r, w_b, dst):
                def f():
                    pm = psum.tile([128, 512], F32, tag="mm", name="pm_qk")
                    for ko in range(KO):
                        nc.tensor.matmul(
                            pm[:],
                            lhsT=w_b[:, ko, :],
                            rhs=xnrs[r][:, ko, :],
                            start=(ko == 0),
                            stop=(ko == KO - 1),
                        )
                    nc.vector.tensor_copy(
                        out=dst[:, r * 512 : (r + 1) * 512], in_=pm[:]
                    )

                return f

            def t_vv(r, lt):
                def f():
                    pv = psum.tile([128, 2 * DH], F32, tag="mm", name="pm_vv")
                    for ko in range(KO):
                        nc.tensor.matmul(
                            pv[:],
                            lhsT=xnrs[r][:, ko, lt * 128 : (lt + 1) * 128],
                            rhs=wv_b[:, ko, :],
                            start=(ko == 0),
                            stop=(ko == KO - 1),
                        )
                    tile_g = 4 * r + lt  # global row tile = 16*b + jt
                    nc.vector.tensor_copy(out=v3d[:, tile_g, :, 0:DH], in_=pv[:])

                return f

            def run_tasks(tasks):
                for f in tasks:
                    f()

            # ---- Phase 4: attention (flash, transposed outputs) ----
            aoT = singles.tile([128, R], BF16, tag="aoT")

            def emit_attention(b, tasks):
                ti = 0

                def pop(k):
                    nonlocal ti
                    for _ in range(k):
                        if ti < len(tasks):
                            tasks[ti]()
                            ti += 1

                for ic4 in range(4):
                    i0 = N * b + 512 * ic4
                    avT = [
                        psum.tile([128, 512], F32, tag=f"av{h}", bufs=1, name=f"avT{h}")
                        for h in range(H_LOC)
                    ]
                    es = {}
                    for jt in range(16):
                        sc = psum.tile([128, 1024], F32, tag="sc", name="sc")
                        for h in range(H_LOC):
                            nc.tensor.matmul(
                                sc[:, h * 512 : (h + 1) * 512],
                                lhsT=kT[
                                    DH * h : DH * (h + 1),
                                    N * b + jt * 128 : N * b + (jt + 1) * 128,
                                ],
                                rhs=qT[DH * h : DH * (h + 1), i0 : i0 + 512],
                                start=True,
                                stop=True,
                            )
                        e = epool.tile([128, 1024], BF16, tag="etile", name="e")
                        nc.scalar.activation(out=e[:], in_=sc[:], func=AF.Exp, scale=SCALE)
                        es[jt] = e
                        if jt >= 1:
                            jp = jt - 1
                            for h in range(H_LOC):
                                nc.tensor.matmul(
                                    avT[h][:],
                                    lhsT=v3d[:, 16 * b + jp, h, :],
                                    rhs=es[jp][:, h * 512 : (h + 1) * 512],
                                    start=(jp == 0),
                                    stop=False,
                                )
                            del es[jp]
                        pop(1)
                    for h in range(H_LOC):
                        nc.tensor.matmul(
                            avT[h][:],
                            lhsT=v3d[:, 16 * b + 15, h, :],
                            rhs=es[15][:, h * 512 : (h + 1) * 512],
                            start=False,
                            stop=True,
                        )
                    del es[15]
                    # drain + normalize: aoT = av / den, via base-0 copies
                    # (reciprocal_approx_fast corrupts on shifted-base APs)
                    for h in range(H_LOC):
                        drain = small.tile([DH, 512], F32, tag="drain", bufs=2, name="drain")
                        nc.vector.tensor_copy(out=drain[:], in_=avT[h][0:DH, :])
                        den0 = small.tile([DH, 512], F32, tag="den0", bufs=2, name="den0")
                        nc.vector.tensor_copy(out=den0[:], in_=avT[h][DH : 2 * DH, :])
                        rec = small.tile([DH, 512], F32, tag="rec", bufs=2, name="rec")
                        nc.vector.reciprocal_approx_fast(out=rec[:], in_=den0[:])
                        nc.vector.tensor_tensor(
                            out=aoT[DH * h : DH * (h + 1), i0 : i0 + 512],
                            in0=drain[:],
                            in1=rec[:],
                            op=ALU.mult,
                        )
                    # eager bounce of this unit's columns (dest core g)
                    g = b * 4 + ic4
                    nc.gpsimd.dma_start(
                        out=ao_bounce[g, :, :], in_=aoT[:, g * 512 : (g + 1) * 512]
                    )
                pop(len(tasks))  # drain leftovers

            # ramp: r0 fully, r1 except q
            run_tasks(
                [t_xnr(0), t_xnr(1), t_kq(0, wk_b, kT), t_kq(0, wq_b, qT)]
                + [t_vv(0, lt) for lt in range(4)]
                + [t_kq(1, wk_b, kT)]
                + [t_vv(1, lt) for lt in range(4)]
            )
            tasks_b0 = (
                [t_xnr(2), t_kq(2, wk_b, kT), t_xnr(3), t_kq(3, wk_b, kT)]
                + [t_kq(1, wq_b, qT)]
                + [t_vv(2, lt) for lt in range(4)]
                + [t_vv(3, lt) for lt in range(4)]
                + [t_kq(2, wq_b, qT), t_kq(3, wq_b, qT)]
                + [t_xnr(4), t_kq(4, wk_b, kT)]
                + [t_vv(4, lt) for lt in range(4)]
                + [t_xnr(5), t_kq(5, wk_b, kT)]
                + [t_vv(5, lt) for lt in range(4)]
                + [t_xnr(6), t_kq(6, wk_b, kT)]
                + [t_vv(6, lt) for lt in range(4)]
                + [t_xnr(7), t_kq(7, wk_b, kT)]
                + [t_vv(7, lt) for lt in range(4)]
                + [t_kq(4, wq_b, qT), t_kq(5, wq_b, qT)]
                + [t_kq(6, wq_b, qT), t_kq(7, wq_b, qT)]
            )
            emit_attention(0, tasks_b0)
            emit_attention(1, [])

            # ---- Phase 5: AllToAll attn_out^T ----
            nc.gpsimd.collective_compute(
                "AllToAll",
                ALU.bypass,
                ins=[ao_bounce[:]],
                outs=[ao_recv[:]],
                replica_groups=GROUPS,
            )

            # keep the PE warm across the AllToAll window
            for _w in range(20):
                pmw = psum.tile([128, 512], F32, tag="mm", name="pm_warm")
                nc.tensor.matmul(
                    pmw[:],
                    lhsT=xnT_sb[:, 0, 0:128],
                    rhs=xnT_sb[:, 1, :],
                    start=True,
                    stop=True,
                )

            # ---- Phase 6: out-projection ----
            aoT3d = singles.tile([128, KO, RL], BF16, tag="aoT3d")
            for mt in range(4):
                eng = nc.scalar if mt % 2 == 0 else nc.gpsimd
                eng.dma_start(
                    out=aoT3d[:, :, mt * 128 : (mt + 1) * 128],
                    in_=ao_recv[:, :, mt * 128 : (mt + 1) * 128].rearrange(
                        "r p lr -> p r lr"
                    ),
                )
            for mt in range(4):
                for n2 in range(2):
                    pm = psum.tile([128, 512], F32, tag="mm", name="pm_out")
                    for ko in range(KO):
                        nc.tensor.matmul(
                            pm[:],
                            lhsT=aoT3d[:, ko, mt * 128 : (mt + 1) * 128],
                            rhs=wo_b[:, ko, n2 * 512 : (n2 + 1) * 512],
                            start=(ko == 0),
                            stop=(ko == KO - 1),
                        )
                    o_sb = temps.tile([128, 512], F32, tag="osb")
                    nc.vector.tensor_tensor(
                        out=o_sb[:],
                        in0=pm[:],
                        in1=bias_bc[:, n2 * 512 : (n2 + 1) * 512],
                        op=ALU.add,
                    )
                    eng = nc.scalar if (mt * 2 + n2) % 2 == 0 else nc.gpsimd
                    eng.dma_start(
                        out=out_ext[
                            mt * 128 : (mt + 1) * 128, n2 * 512 : (n2 + 1) * 512
                        ],
                        in_=o_sb[:],
                    )

    nc.compile()
    return nc


_NC_CACHE = None


def _get_nc():
    global _NC_CACHE
    if _NC_CACHE is None:
        _NC_CACHE = build_nc()
    return _NC_CACHE


def _shard_inputs(x, w_q, w_kv, w_out, b_out, gamma, beta):
    xr = np.ascontiguousarray(x.reshape(R, DIM))
    in_maps = []
    for c in range(N_CORES):
        in_maps.append(
            {
                "x": np.ascontiguousarray(xr[RL * c : RL * (c + 1)]),
                "wq": np.ascontiguousarray(w_q[:, CH * c : CH * (c + 1)]),
                "wk": np.ascontiguousarray(w_kv[:, CH * c : CH * (c + 1)]),
                "wv": np.ascontiguousarray(
                    w_kv[:, DIM + CH * c : DIM + CH * (c + 1)]
                ),
                "wo": np.ascontiguousarray(w_out),
                "gamma": np.ascontiguousarray(gamma),
                "beta": np.ascontiguousarray(beta),
                "bias": np.ascontiguousarray(b_out),
            }
        )
    return in_maps


def run_sharded(x, w_q, w_kv, w_out, b_out, gamma, beta, trace=False, **trace_kwargs):
    nc = _get_nc()
    in_maps = _shard_inputs(
        np.asarray(x, np.float32),
        np.asarray(w_q, np.float32),
        np.asarray(w_kv, np.float32),
        np.asarray(w_out, np.float32),
        np.asarray(b_out, np.float32),
        np.asarray(gamma, np.float32),
        np.asarray(beta, np.float32),
    )
    res = run_bass_kernel_spmd(
        nc, in_maps, core_ids=list(range(N_CORES)), trace=trace, **trace_kwargs
    )
    out = np.concatenate([res.results[c]["out"] for c in range(N_CORES)], axis=0)
    return out.reshape(2, N, DIM), res


def kernel(x, w_q, w_kv, w_out, b_out, gamma, beta):
    out, _ = run_sharded(x, w_q, w_kv, w_out, b_out, gamma, beta, trace=False)
    return out


# revision 14
# speedup vs baseline: 1.0256x; 1.0256x over previous
"""Distributed Trainium2 Bass kernel for nn_Attention (dense transformer block).

Reference computation (full shapes):
    x: [2, 2048, 1024]
    xn = LayerNorm(x, gamma, beta)
    q = xn @ w_q ; k, v = split(xn @ w_kv)   (16 heads, head dim 64)
    attn = softmax(q k^T / 8) v  over seq 2048
    out = attn_out @ w_out + b_out           -> [2, 2048, 1024]

Sharding over 8 NeuronCores (head tensor-parallel, 2 heads/core; rows of the
flattened [4096, 1024] activations sharded 512/core for LayerNorm/out-proj).
All 8 cores run one identical SPMD graph.

v3 design (bf16 numerics; fp8 tested and rejected — each fp8 matmul operand
costs 2.7-4% output rel err vs the 2e-2 gate):
  * Dummy AllGather at t=0 absorbs the NRT first-collective barrier +
    CC-trigger wake latency concurrent with the LayerNorm front-end.
  * x DMAs issued first on the scalar queue; params on gpsimd behind the
    dummy doorbell; weight loads deferred past the real AllGather doorbell.
  * LayerNorm -> PE transpose -> gamma/beta -> xn^T bf16, bounced to DRAM
    per ko-chunk so the AllGather doorbell fires ~17us in.
  * Attention: flash over 16 j-tiles per (batch, 512-i-chunk); both heads'
    score matmuls overlap in the PE array (disjoint row groups); exp on ACT
    straight from PSUM; attn@v with a 64-wide ones block giving denominator
    replicas; softmax normalize via reciprocal_approx_fast (inputs must be
    partition-base-0: custom DVE ops corrupt on shifted bases).
  * QKV for later row-blocks woven into attention emission as fine-grained
    tasks; PSUM drains on DVE (not ACT) to keep the exp stream dense.
  * attn-out columns bounced to DRAM eagerly per unit; AllToAll fires
    immediately after the last unit; out-proj + bias add + store chunked.
"""

import numpy as np

import concourse.bass as bass
import concourse.mybir as mybir
import concourse.tile as tile
from concourse import bacc
from concourse.bass_utils import run_bass_kernel_spmd

F32 = mybir.dt.float32
BF16 = mybir.dt.bfloat16
FP8 = mybir.dt.float8e4
AF = mybir.ActivationFunctionType
ALU = mybir.AluOpType

N_CORES = 8
DIM = 1024
N = 2048  # sequence length
R = 4096  # total rows (2 batches x 2048)
RL = 512  # rows per core
H_LOC = 2  # heads per core
DH = 64
CH = H_LOC * DH  # 128 channels per core
SCALE = DH**-0.5
KO = DIM // 128  # 8 contraction chunks
GROUPS = [list(range(N_CORES))]


def build_nc():
    nc = bacc.Bacc("TRN2", target_bir_lowering=False, debug=False, num_devices=N_CORES)

    x_ext = nc.declare_dram_parameter("x", [RL, DIM], F32, isOutput=False)
    wq_ext = nc.declare_dram_parameter("wq", [DIM, CH], F32, isOutput=False)
    wk_ext = nc.declare_dram_parameter("wk", [DIM, CH], F32, isOutput=False)
    wv_ext = nc.declare_dram_parameter("wv", [DIM, CH], F32, isOutput=False)
    wo_ext = nc.declare_dram_parameter("wo", [DIM, DIM], F32, isOutput=False)
    gamma_ext = nc.declare_dram_parameter("gamma", [DIM], F32, isOutput=False)
    beta_ext = nc.declare_dram_parameter("beta", [DIM], F32, isOutput=False)
    bias_ext = nc.declare_dram_parameter("bias", [DIM], F32, isOutput=False)
    out_ext = nc.declare_dram_parameter("out", [RL, DIM], F32, isOutput=True)

    # DRAM bounce buffers for collectives
    dummy_in = nc.dram_tensor("dummy_in", [64], FP8)
    dummy_out = nc.dram_tensor("dummy_out", [N_CORES * 64], FP8, addr_space="Shared")
    xnT_bounce = nc.dram_tensor("xnT_bounce", [DIM, RL], BF16)
    xnT_gath = nc.dram_tensor(
        "xnT_gath", [N_CORES * DIM, RL], BF16, addr_space="Shared"
    )
    ao_bounce = nc.dram_tensor("ao_bounce", [N_CORES, CH, RL], BF16)
    ao_recv = nc.dram_tensor("ao_recv", [N_CORES, CH, RL], BF16)

    with tile.TileContext(nc) as tc:
        with (
            tc.tile_pool(name="singles", bufs=1) as singles,
            tc.tile_pool(name="temps", bufs=3) as temps,
            tc.tile_pool(name="small", bufs=4) as small,
            tc.tile_pool(name="etile", bufs=3) as epool,
            tc.tile_pool(name="xnrp", bufs=8) as xnrp,
            tc.tile_pool(name="psum", bufs=2, space="PSUM") as psum,
        ):
            # ---- x DMAs first on scalar: LayerNorm input is the critical path
            x_tiles = []
            for t in range(4):
                x_t = temps.tile([128, DIM], F32, tag="x", bufs=4)
                nc.scalar.dma_start(out=x_t[:], in_=x_ext[t * 128 : (t + 1) * 128, :])
                x_tiles.append(x_t)

            # constants / params on gpsimd (behind the dummy doorbell)
            import ml_dtypes

            ident_const = nc.inline_tensor(
                np.eye(128, dtype=ml_dtypes.bfloat16), name="ident_const"
            )
            ident = singles.tile([128, 128], BF16, tag="ident")
            nc.gpsimd.dma_start(out=ident[:], in_=ident_const.ap())
            gamma_sb = singles.tile([128, KO], F32, tag="gamma")
            nc.gpsimd.dma_start(
                out=gamma_sb[:], in_=gamma_ext.ap().rearrange("(ko p) -> p ko", p=128)
            )
            beta_sb = singles.tile([128, KO], F32, tag="beta")
            nc.gpsimd.dma_start(
                out=beta_sb[:], in_=beta_ext.ap().rearrange("(ko p) -> p ko", p=128)
            )
            eps_sb = singles.tile([128, 1], F32, tag="eps")
            nc.vector.memset(eps_sb[:], 1e-5)

            # ---- Phase 1: LayerNorm on own 512 rows -> xc bf16 ----
            xc = []
            for t in range(4):
                x_t = x_tiles[t]
                st6 = small.tile([128, 2, 6], F32, tag="st6")
                nc.vector.bn_stats(out=st6[:, 0, :], in_=x_t[:, 0:512])
                nc.vector.bn_stats(out=st6[:, 1, :], in_=x_t[:, 512:1024])
                mv = small.tile([128, 2], F32, tag="mv")
                nc.vector.bn_aggr(out=mv[:], in_=st6[:])
                sd = small.tile([128, 1], F32, tag="sd")
                nc.scalar.activation(
                    out=sd[:], in_=mv[:, 1:2], func=AF.Sqrt, bias=eps_sb[:], scale=1.0
                )
                istd = small.tile([128, 1], F32, tag="istd")
                nc.vector.reciprocal_approx_fast(out=istd[:], in_=sd[:])
                xc_t = singles.tile([128, DIM], BF16, tag=f"xc{t}", name=f"xc{t}")
                for hh in range(2):
                    nc.vector.tensor_scalar(
                        xc_t[:, hh * 512 : (hh + 1) * 512],
                        x_t[:, hh * 512 : (hh + 1) * 512],
                        mv[:, 0:1],
                        istd[:],
                        ALU.subtract,
                        ALU.mult,
                    )
                xc.append(xc_t)

            # ---- transpose + gamma/beta -> xn^T bf16; bounce per ko chunk ----
            xnT_sb = singles.tile([128, KO, RL], BF16, tag="xnT")
            for ko in range(KO):
                ptr = psum.tile([128, 4, 128], BF16, tag="mm")
                for t in range(4):
                    nc.tensor.transpose(
                        ptr[:, t, :], xc[t][:, ko * 128 : (ko + 1) * 128], ident[:]
                    )
                nc.vector.tensor_scalar(
                    xnT_sb[:, ko, :],
                    ptr.rearrange("p a b -> p (a b)"),
                    gamma_sb[:, ko : ko + 1],
                    beta_sb[:, ko : ko + 1],
                    ALU.mult,
                    ALU.add,
                )
                nc.scalar.dma_start(
                    out=xnT_bounce[ko * 128 : (ko + 1) * 128, :], in_=xnT_sb[:, ko, :]
                )

            # ---- Phase 2: AllGather xn^T (all 8 cores) ----
            nc.gpsimd.collective_compute(
                "AllGather",
                ALU.bypass,
                ins=[xnT_bounce[:]],
                outs=[xnT_gath[:]],
                replica_groups=GROUPS,
            )

            # ---- weight loads + bf16 casts during the AllGather window ----
            def load_weight_bf16(ext, cols, tag):
                wf = singles.tile([128, KO, cols], F32, tag="wf", name="wf")
                nc.scalar.dma_start(
                    out=wf[:], in_=ext.ap().rearrange("(ko p) m -> p ko m", p=128)
                )
                wb = singles.tile([128, KO, cols], BF16, tag=tag, name=tag)
                nc.scalar.activation(
                    out=wb.rearrange("p a b -> p (a b)"),
                    in_=wf.rearrange("p a b -> p (a b)"),
                    func=AF.Copy,
                )
                return wb

            wq_b = load_weight_bf16(wq_ext, CH, "wq")
            wk_b = load_weight_bf16(wk_ext, CH, "wk")
            wv_b = load_weight_bf16(wv_ext, CH, "wv")

            # preload the exp ACT table while the scalar engine is idle
            dume = small.tile([128, 1], F32, tag="dume")
            nc.scalar.activation(out=dume[:], in_=eps_sb[:], func=AF.Exp, scale=1.0)

            # w_out: gpsimd stages f32, vector casts to bf16
            wo_b = singles.tile([128, KO, DIM], BF16, tag="wo", name="wo")
            for ko in range(KO):
                wof = temps.tile([128, DIM], F32, tag="x", bufs=4, name="wstage")
                nc.gpsimd.dma_start(
                    out=wof[:], in_=wo_ext[ko * 128 : (ko + 1) * 128, :]
                )
                nc.vector.tensor_copy(out=wo_b[:, ko, :], in_=wof[:])

            # bias broadcast [128,DIM] (partition-stride-0 read from DRAM)
            bias_bc = singles.tile([128, DIM], F32, tag="bias_bc")
            nc.gpsimd.dma_start(
                out=bias_bc[:],
                in_=bass.AP(tensor=bias_ext, offset=0, ap=[[0, 128], [1, DIM]]),
            )

            # v3d: stationary operand for attn@v. [128p, 32 rowtiles, 2h, 128]:
            # d 0:64 = v rows, d 64:128 = ones (64 denominator replicas).
            v3d = singles.tile([128, 32, H_LOC, 2 * DH], BF16, tag="v3d")
            nc.vector.memset(v3d[:, :, :, DH : 2 * DH], 1.0)

            # keep PE clocked across the AllGather window (results unused)
            for _w in range(40):
                pmw = psum.tile([128, 512], F32, tag="mm", name="pm_warm_ag")
                nc.tensor.matmul(
                    pmw[:],
                    lhsT=xnT_sb[:, 0, 0:128],
                    rhs=xnT_sb[:, 1, :],
                    start=True,
                    stop=True,
                )

            # ---- Phase 3: QKV (own 2 heads) as weavable tasks ----
            qT = singles.tile([128, R], BF16, tag="qT")
            kT = singles.tile([128, R], BF16, tag="kT")
            xnrs = {}

            def t_xnr(r):
                def f():
                    xnr = xnrp.tile([128, KO, RL], BF16, tag="xnr", name="xnr")
                    nc.gpsimd.dma_start(
                        out=xnr[:],
                        in_=xnT_gath[r * DIM : (r + 1) * DIM, :].rearrange(
                            "(ko p) lr -> p ko lr", p=128
                        ),
                    )
                    xnrs[r] = xnr

                return f

            def t_kq(r, w_b, dst):
                def f():
                    pm = psum.tile([128, 512], F32, tag="mm", name="pm_qk")
                    for ko in range(KO):
                        nc.tensor.matmul(
                            pm[:],
                            lhsT=w_b[:, ko, :],
                            rhs=xnrs[r][:, ko, :],
                            start=(ko == 0),
                            stop=(ko == KO - 1),
                        )
                    nc.vector.tensor_copy(
                        out=dst[:, r * 512 : (r + 1) * 512], in_=pm[:]
                    )

                return f

            def t_vv(r, lt):
                def f():
                    pv = psum.tile([128, 2 * DH], F32, tag="mm", name="pm_vv")
                    for ko in range(KO):
                        nc.tensor.matmul(
                            pv[:],
                            lhsT=xnrs[r][:, ko, lt * 128 : (lt + 1) * 128],
                            rhs=wv_b[:, ko, :],
                            start=(ko == 0),
                            stop=(ko == KO - 1),
                        )
                    tile_g = 4 * r + lt  # global row tile = 16*b + jt
                    nc.vector.tensor_copy(out=v3d[:, tile_g, :, 0:DH], in_=pv[:])

                return f

            def run_tasks(tasks):
                for f in tasks:
                    f()

            # ---- Phase 4: attention (flash, transposed outputs) ----
            aoT = singles.tile([128, R], BF16, tag="aoT")

            def emit_attention(b, tasks):
                ti = 0

                def pop(k):
                    nonlocal ti
                    for _ in range(k):
                        if ti < len(tasks):
                            tasks[ti]()
                            ti += 1

                for ic4 in range(4):
                    i0 = N * b + 512 * ic4
                    avT = [
                        psum.tile([128, 512], F32, tag=f"av{h}", bufs=1, name=f"avT{h}")
                        for h in range(H_LOC)
                    ]
                    es = {}
                    for jt in range(16):
                        sc = psum.tile([128, 1024], F32, tag="sc", name="sc")
                        for h in range(H_LOC):
                            nc.tensor.matmul(
                                sc[:, h * 512 : (h + 1) * 512],
                                lhsT=kT[
                                    DH * h : DH * (h + 1),
                                    N * b + jt * 128 : N * b + (jt + 1) * 128,
                                ],
                                rhs=qT[DH * h : DH * (h + 1), i0 : i0 + 512],
                                start=True,
                                stop=True,
                            )
                        e = epool.tile([128, 1024], BF16, tag="etile", name="e")
                        nc.scalar.activation(out=e[:], in_=sc[:], func=AF.Exp, scale=SCALE)
                        es[jt] = e
                        if jt >= 1:
                            jp = jt - 1
                            for h in range(H_LOC):
                                nc.tensor.matmul(
                                    avT[h][:],
                                    lhsT=v3d[:, 16 * b + jp, h, :],
                                    rhs=es[jp][:, h * 512 : (h + 1) * 512],
                                    start=(jp == 0),
                                    stop=False,
                                )
                            del es[jp]
                        pop(1)
                    for h in range(H_LOC):
                        nc.tensor.matmul(
                            avT[h][:],
                            lhsT=v3d[:, 16 * b + 15, h, :],
                            rhs=es[15][:, h * 512 : (h + 1) * 512],
                            start=False,
                            stop=True,
                        )
                    del es[15]
                    # drain + normalize: aoT = av / den, via base-0 copies
                    # (reciprocal_approx_fast corrupts on shifted-base APs)
                    for h in range(H_LOC):
                        drain = small.tile([DH, 512], F32, tag="drain", bufs=2, name="drain")
                        nc.vector.tensor_copy(out=drain[:], in_=avT[h][0:DH, :])
                        den0 = small.tile([DH, 512], F32, tag="den0", bufs=2, name="den0")
                        nc.vector.tensor_copy(out=den0[:], in_=avT[h][DH : 2 * DH, :])
                        rec = small.tile([DH, 512], F32, tag="rec", bufs=2, name="rec")
                        nc.vector.reciprocal_approx_fast(out=rec[:], in_=den0[:])
                        nc.vector.tensor_tensor(
                            out=aoT[DH * h : DH * (h + 1), i0 : i0 + 512],
                            in0=drain[:],
                            in1=rec[:],
                            op=ALU.mult,
                        )
                    # eager bounce of this unit's columns (dest core g)
                    g = b * 4 + ic4
                    nc.gpsimd.dma_start(
                        out=ao_bounce[g, :, :], in_=aoT[:, g * 512 : (g + 1) * 512]
                    )
                pop(len(tasks))  # drain leftovers

            # ramp: r0 fully, r1 except q
            run_tasks(
                [t_xnr(0), t_xnr(1), t_kq(0, wk_b, kT), t_kq(0, wq_b, qT)]
                + [t_vv(0, lt) for lt in range(4)]
                + [t_kq(1, wk_b, kT)]
                + [t_vv(1, lt) for lt in range(4)]
            )
            tasks_b0 = (
                [t_xnr(2), t_kq(2, wk_b, kT), t_xnr(3), t_kq(3, wk_b, kT)]
                + [t_kq(1, wq_b, qT)]
                + [t_vv(2, lt) for lt in range(4)]
                + [t_vv(3, lt) for lt in range(4)]
                + [t_kq(2, wq_b, qT), t_kq(3, wq_b, qT)]
                + [t_xnr(4), t_kq(4, wk_b, kT)]
                + [t_vv(4, lt) for lt in range(4)]
                + [t_xnr(5), t_kq(5, wk_b, kT)]
                + [t_vv(5, lt) for lt in range(4)]
                + [t_xnr(6), t_kq(6, wk_b, kT)]
                + [t_vv(6, lt) for lt in range(4)]
                + [t_xnr(7), t_kq(7, wk_b, kT)]
                + [t_vv(7, lt) for lt in range(4)]
                + [t_kq(4, wq_b, qT), t_kq(5, wq_b, qT)]
                + [t_kq(6, wq_b, qT), t_kq(7, wq_b, qT)]
            )
            emit_attention(0, tasks_b0)
            emit_attention(1, [])

            # ---- Phase 5: AllToAll attn_out^T ----
            nc.gpsimd.collective_compute(
                "AllToAll",
                ALU.bypass,
                ins=[ao_bounce[:]],
                outs=[ao_recv[:]],
                replica_groups=GROUPS,
            )

            # keep the PE warm across the AllToAll window
            for _w in range(20):
                pmw = psum.tile([128, 512], F32, tag="mm", name="pm_warm")
                nc.tensor.matmul(
                    pmw[:],
                    lhsT=xnT_sb[:, 0, 0:128],
                    rhs=xnT_sb[:, 1, :],
                    start=True,
                    stop=True,
                )

            # ---- Phase 6: out-projection ----
            # 8 contiguous per-source loads (a strided p<->r transpose DMA
            # would gather in 256B runs and crawl)
            ao_r = singles.tile([128, KO, RL], BF16, tag="aoT3d")
            for r in range(KO):
                eng = nc.scalar if r % 2 == 0 else nc.gpsimd
                eng.dma_start(out=ao_r[:, r, :], in_=ao_recv[r, :, :])
            for mt in range(4):
                for n2 in range(2):
                    pm = psum.tile([128, 512], F32, tag="mm", name="pm_out")
                    for ko in range(KO):
                        nc.tensor.matmul(
                            pm[:],
                            lhsT=ao_r[:, ko, mt * 128 : (mt + 1) * 128],
                            rhs=wo_b[:, ko, n2 * 512 : (n2 + 1) * 512],
                            start=(ko == 0),
                            stop=(ko == KO - 1),
                        )
                    o_sb = temps.tile([128, 512], F32, tag="osb")
                    nc.vector.tensor_tensor(
                        out=o_sb[:],
                        in0=pm[:],
                        in1=bias_bc[:, n2 * 512 : (n2 + 1) * 512],
                        op=ALU.add,
                    )
                    eng = nc.scalar if (mt * 2 + n2) % 2 == 0 else nc.gpsimd
                    eng.dma_start(
                        out=out_ext[
                            mt * 128 : (mt + 1) * 128, n2 * 512 : (n2 + 1) * 512
                        ],
                        in_=o_sb[:],
                    )

    nc.compile()
    return nc


_NC_CACHE = None


def _get_nc():
    global _NC_CACHE
    if _NC_CACHE is None:
        _NC_CACHE = build_nc()
    return _NC_CACHE


def _shard_inputs(x, w_q, w_kv, w_out, b_out, gamma, beta):
    xr = np.ascontiguousarray(x.reshape(R, DIM))
    in_maps = []
    for c in range(N_CORES):
        in_maps.append(
            {
                "x": np.ascontiguousarray(xr[RL * c : RL * (c + 1)]),
                "wq": np.ascontiguousarray(w_q[:, CH * c : CH * (c + 1)]),
                "wk": np.ascontiguousarray(w_kv[:, CH * c : CH * (c + 1)]),
                "wv": np.ascontiguousarray(
                    w_kv[:, DIM + CH * c : DIM + CH * (c + 1)]
                ),
                "wo": np.ascontiguousarray(w_out),
                "gamma": np.ascontiguousarray(gamma),
                "beta": np.ascontiguousarray(beta),
                "bias": np.ascontiguousarray(b_out),
            }
        )
    return in_maps


def run_sharded(x, w_q, w_kv, w_out, b_out, gamma, beta, trace=False, **trace_kwargs):
    nc = _get_nc()
    in_maps = _shard_inputs(
        np.asarray(x, np.float32),
        np.asarray(w_q, np.float32),
        np.asarray(w_kv, np.float32),
        np.asarray(w_out, np.float32),
        np.asarray(b_out, np.float32),
        np.asarray(gamma, np.float32),
        np.asarray(beta, np.float32),
    )
    res = run_bass_kernel_spmd(
        nc, in_maps, core_ids=list(range(N_CORES)), trace=trace, **trace_kwargs
    )
    out = np.concatenate([res.results[c]["out"] for c in range(N_CORES)], axis=0)
    return out.reshape(2, N, DIM), res


def kernel(x, w_q, w_kv, w_out, b_out, gamma, beta):
    out, _ = run_sharded(x, w_q, w_kv, w_out, b_out, gamma, beta, trace=False)
    return out
